# revision 49
# baseline (speedup 1.0000x reference)
r"""ALiBi multi-head causal attention on 8 TRN2 NeuronCores.

Problem: B=2, S=2048, E=2048, H=16, D=128, f32 I/O.

Sharding: each core owns 2 heads (core c -> heads {2c, 2c+1}) for BOTH
batches. Per core: QKV projections for its heads, causal attention with
ALiBi, then one 8-core AllToAll per local head re-shards from head-split
to row-split (the first collective overlaps the second head's attention,
the second overlaps the first half of the output projection), and each
core computes the output projection for its (batch, row-slice):
core c -> batch c//4, rows (c%4)*512 .. +512.

Compute dtype bf16 (fp32 PSUM accumulation). Numerical scheme for the
softmax (scores are built TRANSPOSED: ST[sj, si] so the P@V contraction
needs no on-chip transpose of P):

  exp argument = scale*qk[sj,si] + slope*sj  + scale*brow[si]
                 \__ matmul __/   \_ACT bias_/  \_ K=128 matmul _/

where brow[si] = bf16(-slope*si/scale) is replicated across partitions
and added via a K=128 matmul against a constant 1/128 stationary operand
(exact, and keeps the PE MAC-dense so the HAM clock gate stays open).

The row max is never computed: with x ~ N(0,1) and W ~ N(0,1/E) the raw
qk*scale scores are bounded (~±60), so exp never overflows fp32/bf16, and
the per-row (si) shift -slope*si only needs to be applied approximately:
any per-si error is a multiplicative per-row factor that cancels exactly
in the softmax normalization (rowsum is obtained via a ones-column
appended to V). The causal mask is a -1e9 additive [128,128] triangle on
the diagonal tiles; fully-masked tiles are simply never computed.

Schedule: score strips (ACT-bound) are interleaved at matmul granularity
with the previous iteration's P@V groups (PE-dense); normalized O tiles
are staged straight into the A2A input buffers (one DMA per row tile);
the transposed ZT loads ([si,d] -> [d,si] via the DMA xbar) are pinned
behind the staging DMAs so a collective-wait never blocks the sync queue.
"""

import math

import numpy as np
import ml_dtypes

import concourse.bass as bass
import concourse.tile as tile
from concourse.tile import add_dep_helper
from concourse import bacc, mybir
from concourse.bass_utils import run_bass_kernel_spmd

BF16 = mybir.dt.bfloat16
F32 = mybir.dt.float32
NP_BF16 = ml_dtypes.bfloat16

B, S, E, H, D = 2, 2048, 2048, 16, 128
N_CORES = 8
HPC = H // N_CORES  # heads per core = 2
SCALE = 1.0 / math.sqrt(D)
ET = E // 128  # 16 e-tiles
ST_TILES = S // 128  # 16 sequence tiles
NEG = -1.0e9


def _slopes():
    # H=16 is a power of two: slope_i = 2^(-(i+1)/2)
    start = 2.0 ** (-(2.0 ** -(math.log2(H) - 3)))
    return [start * start**i for i in range(H)]


def _build():
    nc = bacc.Bacc("TRN2", target_bir_lowering=False, debug=False,
                   num_devices=N_CORES)

    xT = [nc.dram_tensor(f"xT{b}", [E, S], BF16, kind="ExternalInput")
          for b in range(B)]
    wq = nc.dram_tensor("wq", [E, HPC * D], BF16, kind="ExternalInput")
    wk = nc.dram_tensor("wk", [E, HPC * D], BF16, kind="ExternalInput")
    wv = nc.dram_tensor("wv", [E, HPC * D], BF16, kind="ExternalInput")
    woT = nc.dram_tensor("woT", [E, E], BF16, kind="ExternalInput")
    bias_sj = nc.dram_tensor("bias_sj", [128, HPC * ST_TILES], F32,
                             kind="ExternalInput")
    brow = nc.dram_tensor("brow", [1, HPC * S], BF16, kind="ExternalInput")
    diag = nc.dram_tensor("diag", [128, 128], F32, kind="ExternalInput")
    out_ext = nc.dram_tensor("out", [512, E], F32, kind="ExternalOutput")

    # A2A (one per local head hh, so the first collective overlaps the
    # second head's attention): chunk j (sent to core j) = head hh's
    # normalized attention output [512(si), 128(d)] for core j's
    # (batch j//4, rows (j%4)*512..+512). After A2A, chunk j = head
    # 2j+hh for MY (batch, rows) slice; transposed to [d, si] by the
    # xbar on the DRAM->SBUF load.
    a2a_in = [nc.dram_tensor(f"a2a_in{h}", [N_CORES, 512, D], BF16)
              for h in range(HPC)]
    a2a_out = [nc.dram_tensor(f"a2a_out{h}", [N_CORES, 512, D], BF16)
               for h in range(HPC)]

    with tile.TileContext(nc) as tc:
        _emit(nc, tc, xT, wq, wk, wv, woT, bias_sj, brow, diag,
              a2a_in, a2a_out, out_ext)

    nc.compile()
    return nc


def _emit(nc, tc, xT, wq, wk, wv, woT, bias_sj, brow, diag,
          a2a_in, a2a_out, out_ext):
    from contextlib import ExitStack

    with ExitStack() as top:
        consts = top.enter_context(tc.tile_pool(name="consts", bufs=1))
        qkv = top.enter_context(tc.tile_pool(name="qkv", bufs=1))
        proj_psum = top.enter_context(
            tc.tile_pool(name="proj_psum", bufs=3, space="PSUM"))

        # ---- constants ----
        sb_bias = consts.tile([128, HPC * ST_TILES], F32, tag="bias")
        nc.gpsimd.dma_start(out=sb_bias[:], in_=bias_sj[:])
        # brow replicated across all 128 partitions: the per-si ALiBi shift
        # is added via a K=128 matmul with a constant 1/128 stationary
        # operand (exact: 1/128 is a power of two) -- same cycle count as
        # a rank-1 but with full MAC activity so the HAM clock gate stays
        # released during the score phase.
        sb_brow = consts.tile([128, HPC * S], BF16, tag="brow")
        nc.gpsimd.dma_start(
            out=sb_brow[:],
            in_=bass.AP(tensor=brow.ap().tensor, offset=0,
                        ap=[[0, 128], [1, HPC * S]]))
        sb_diag = consts.tile([128, 128], F32, tag="diag")
        nc.gpsimd.dma_start(out=sb_diag[:], in_=diag[:])
        sb_ones = consts.tile([128, 128], BF16, tag="ones")
        nc.vector.memset(sb_ones[:], 1.0 / 128.0)

        # ---- persistent QKV buffers ----
        # QT/KT: [128(d), B, HPC, S];  V: [128(s), ST_TILES, B, HPC, 132]
        QT = qkv.tile([128, B, HPC, S], BF16, tag="QT")
        KT = qkv.tile([128, B, HPC, S], BF16, tag="KT")
        V = qkv.tile([128, ST_TILES, B, HPC, 132], BF16, tag="V")
        nc.vector.memset(V[:, :, :, :, 128:129], 1.0)  # rowsum ones column

        # ---- phase 1: QKV projections ----
        with tc.tile_pool(name="wbuf", bufs=1) as wbuf, \
             tc.tile_pool(name="xtp", bufs=3) as xtp:
            w_sb = {}
            for name, wt in (("q", wq), ("k", wk), ("v", wv)):
                w_sb[name] = wbuf.tile([128, ET, HPC * D], BF16,
                                       tag=f"w{name}", name=f"w{name}")
            w_r = {name: wt.ap().rearrange("(t p) c -> p t c", p=128)
                   for name, wt in (("q", wq), ("k", wk), ("v", wv))}
            xT_r0 = xT[0].ap().rearrange("(t p) s -> p t s", p=128)
            xt_first = xtp.tile([128, ET, 512], BF16, name="xt_t")
            # head of the DMA queue: interleave wq with the first x
            # chunk so the very first matmul group can start ~6us in
            for q in range(8):
                w = 2
                nc.sync.dma_start(
                    out=w_sb["q"][:, w * q:w * (q + 1), :],
                    in_=w_r["q"][:, w * q:w * (q + 1), :])
                nc.sync.dma_start(
                    out=xt_first[:, w * q:w * (q + 1), :],
                    in_=xT_r0[:, w * q:w * (q + 1), 0:512])
            for name in ("k", "v"):
                for q in range(4):
                    nc.sync.dma_start(
                        out=w_sb[name][:, 4 * q:4 * (q + 1), :],
                        in_=w_r[name][:, 4 * q:4 * (q + 1), :])

            for b in range(B):
                xT_r = xT[b].ap().rearrange("(t p) s -> p t s", p=128)
                for sc in range(4):  # s-chunks of 512
                    if (b, sc) == (0, 0):
                        xt_t = xt_first
                    else:
                        xt_t = xtp.tile([128, ET, 512], BF16, name="xt_t")
                        for q in range(2):
                            w = ET // 2
                            nc.sync.dma_start(
                                out=xt_t[:, w * q:w * (q + 1), :],
                                in_=xT_r[:, w * q:w * (q + 1),
                                         sc * 512:(sc + 1) * 512])
                    # QT / KT (transposed layout; m = head-dim)
                    for name, OUT in (("q", QT), ("k", KT)):
                        for hh in range(HPC):
                            ps = proj_psum.tile([128, 512], F32, tag="ps")
                            for et in range(ET):
                                nc.tensor.matmul(
                                    ps[:],
                                    lhsT=w_sb[name][:, et,
                                                    hh * 128:(hh + 1) * 128],
                                    rhs=xt_t[:, et, :],
                                    start=(et == 0), stop=(et == ET - 1))
                            nc.vector.tensor_copy(
                                out=OUT[:, b, hh, sc * 512:(sc + 1) * 512],
                                in_=ps[:])
                    # V (natural layout; m = sequence)
                    for mt in range(4):
                        ps = proj_psum.tile([128, 512], F32, tag="ps")
                        for et in range(ET):
                            nc.tensor.matmul(
                                ps[:, :HPC * D],
                                lhsT=xt_t[:, et, mt * 128:(mt + 1) * 128],
                                rhs=w_sb["v"][:, et, :],
                                start=(et == 0), stop=(et == ET - 1))
                        st = sc * 4 + mt
                        nc.vector.tensor_copy(
                            out=V[:, st, b, :, 0:128],
                            in_=ps[:, :HPC * D].rearrange(
                                "p (h d) -> p h d", h=HPC))

        # ---- phase 2: attention ----
        with tc.tile_pool(name="ptp", bufs=30) as ptp, \
             tc.tile_pool(name="onp", bufs=8) as onp, \
             tc.tile_pool(name="rcp", bufs=8) as rcp, \
             tc.tile_pool(name="wob", bufs=1) as wob, \
             tc.tile_pool(name="ztb", bufs=1) as ztb, \
             tc.tile_pool(name="y0b", bufs=1) as y0b, \
             tc.tile_pool(name="outp", bufs=3) as outp, \
             tc.tile_pool(name="st_psum", bufs=3, space="PSUM") as st_psum, \
             tc.tile_pool(name="o_psum", bufs=2, space="PSUM") as o_psum:

            ZT = ztb.tile([128, HPC, N_CORES, 512], BF16)
            Y0 = y0b.tile([128, 4, E], BF16)  # even-head partial out proj
            wo_sb = wob.tile([128, ET, E], BF16)
            woT_r = woT.ap().rearrange("(t p) e -> p t e", p=128)
            for q in range(4):  # bulk load off the latency-critical queue
                nc.gpsimd.dma_start(
                    out=wo_sb[:, 4 * q:4 * (q + 1), :],
                    in_=woT_r[:, 4 * q:4 * (q + 1), :])

            def emit_one_strip(b, hh, sb, tj):
                si_lo = max(sb * 512, tj * 128)
                n = (sb + 1) * 512 - si_lo
                ps = st_psum.tile([128, 512], F32, tag="st")
                nc.tensor.matmul(
                    ps[:, :n],
                    lhsT=KT[:, b, hh, tj * 128:(tj + 1) * 128],
                    rhs=QT[:, b, hh, si_lo:si_lo + n],
                    start=True, stop=False)
                nc.tensor.matmul(
                    ps[:, :n],
                    lhsT=sb_ones[:, :],
                    rhs=sb_brow[:, hh * S + si_lo:hh * S + si_lo + n],
                    start=False, stop=True)  # adds brow (K=128, x 1/128)
                if tj >= 4 * sb:  # diagonal tile: causal mask
                    nc.vector.tensor_add(
                        ps[:, 0:128], ps[:, 0:128], sb_diag[:])
                strip = ptp.tile([128, 512], BF16)
                last_exp[0] = nc.scalar.activation(
                    strip[:, :n], ps[:, :n],
                    mybir.ActivationFunctionType.Exp,
                    bias=sb_bias[:, hh * ST_TILES + tj:
                                 hh * ST_TILES + tj + 1],
                    scale=SCALE)
                return (si_lo, strip)

            def pv_gen(hh, b, sb, strips):
                """Yield after each PV matmul; norm + stage between."""
                for ti in range(4 * sb, 4 * sb + 4):
                    op = o_psum.tile([128, 132], F32, name="op")
                    for tj in range(ti + 1):
                        si_lo, strip = strips[tj]
                        col = ti * 128 - si_lo
                        nc.tensor.matmul(
                            op[:, 0:129],
                            lhsT=strip[:, col:col + 128],
                            rhs=V[:, tj, b, hh, 0:129],
                            start=(tj == 0), stop=(tj == ti))
                        yield
                    recip = rcp.tile([128, 1], F32, name="recip")
                    nc.vector.reciprocal(recip[:], op[:, 128:129])
                    onorm = onp.tile([128, 128], BF16, name="onorm")
                    nc.vector.tensor_scalar_mul(
                        onorm[:], op[:, 0:128], recip[:])
                    # stage this row-tile of the A2A chunk for core 4*b+sb
                    dst = a2a_in[hh][4 * b + sb,
                                     (ti % 4) * 128:(ti % 4 + 1) * 128, :]
                    last_stage[0] = nc.sync.dma_start(out=dst, in_=onorm[:])
                    yield

            def emit_a2a(hh):
                nc.gpsimd.collective_compute(
                    "AllToAll",
                    mybir.AluOpType.bypass,
                    ins=[a2a_in[hh].ap().opt()],
                    outs=[a2a_out[hh].ap().opt()],
                    replica_groups=[list(range(N_CORES))],
                )

            def emit_zt(hh, anchor):
                # xbar-transposing load ([si, d] -> [d, si]). Pinned after
                # `anchor`: a collective-wait sitting mid-queue would stall
                # the latency-critical staging DMAs behind it. NOTE: all
                # transposing DMAs stay on the single sync queue -- issuing
                # them concurrently from two HWDGE queues was observed to
                # corrupt data (the xbar-mode serialization that protects
                # DMATranspose/DMACopy transitions is per-queue only).
                inst = nc.sync.dma_start(
                    out=ZT[:, hh, :, :],
                    in_=a2a_out[hh].ap().rearrange("j s d -> (j s) d"),
                    transpose=True)
                if anchor is not None:
                    add_dep_helper(inst.ins, anchor.ins, sync=False,
                                   reason="zt ordering")
                return inst

            def emit_outproj(rnd):
                # rnd 0: even c-tiles (head hh=0 of each core) -> Y0;
                # runs while the second head's collectives are in flight.
                # rnd 1: odd c-tiles + Y0 -> out; mt-outer so row tile mt
                # only waits for si-quarter mt of the A2A.
                cts = [ct for ct in range(ET) if ct % HPC == rnd]
                loop = ([(ec, mt) for ec in range(4) for mt in range(4)]
                        if rnd == 0 else
                        [(ec, mt) for mt in range(4) for ec in range(4)])
                for ec, mt in loop:
                    if True:
                        ps = proj_psum.tile([128, 512], F32, name="ps3", tag="ps")
                        for k, ct in enumerate(cts):
                            nc.tensor.matmul(
                                ps[:],
                                lhsT=ZT[:, ct % HPC, ct // HPC,
                                        mt * 128:(mt + 1) * 128],
                                rhs=wo_sb[:, ct, ec * 512:(ec + 1) * 512],
                                start=(k == 0), stop=(k == len(cts) - 1))
                        if rnd == 0:
                            last_y0[0] = nc.scalar.copy(
                                out=Y0[:, mt, ec * 512:(ec + 1) * 512],
                                in_=ps[:])
                        else:
                            ot = outp.tile([128, 512], F32, name="ot")
                            nc.vector.tensor_add(
                                ot[:], ps[:],
                                Y0[:, mt, ec * 512:(ec + 1) * 512])
                            nc.sync.dma_start(
                                out=out_ext[mt * 128:(mt + 1) * 128,
                                            ec * 512:(ec + 1) * 512],
                                in_=ot[:])

            # software pipeline: iteration i's score strips (ACT-bound,
            # PE-light) are interleaved at matmul granularity with
            # iteration i-1's PV matmuls (PE-dense), so the PE never
            # crawls at the exp rate and the HAM clock gate stays warm.
            iters = [(hh, b, sb)
                     for hh in range(HPC) for b in range(B)
                     for sb in range(4)]
            last_stage = [None]
            zt0_anchor = [None]
            last_exp = [None]
            last_y0 = [None]
            pending = None  # (hh, b, sb, generator)
            for hh, b, sb in iters:
                nstrips = 4 * sb + 4
                strips = {}
                if pending is not None:
                    phh, pb, psb, pgen = pending
                    # prev iteration has 16*psb+14 pv ops; spread them
                    per = (16 * psb + 14 + nstrips - 1) // nstrips + 2
                for tj in range(nstrips):
                    strips[tj] = emit_one_strip(b, hh, sb, tj)
                    if pending is not None:
                        for _ in range(per):
                            if next(pgen, "done") == "done":
                                break
                if pending is not None:
                    for _ in pgen:  # flush remainder
                        pass
                    if pb == B - 1 and psb == 3:
                        emit_a2a(phh)
                    if phh == HPC - 1 and pb == B - 1 and psb == 2:
                        zt0_anchor[0] = last_stage[0]
                pending = (hh, b, sb, pv_gen(hh, b, sb, strips))
            phh, pb, psb, pgen = pending
            for _ in pgen:
                pass
            emit_a2a(phh)

            # ---- phase 3: output projection (split rounds) ----
            zt0 = emit_zt(0, last_stage[0])
            emit_outproj(0)
            emit_zt(1, zt0)
            emit_outproj(1)


_NC_CACHE = None


def _get_nc():
    global _NC_CACHE
    if _NC_CACHE is None:
        _NC_CACHE = _build()
    return _NC_CACHE


def _make_in_maps(x, Wq, Wk, Wv, Wo):
    slopes = _slopes()
    xT = [np.ascontiguousarray(x[b].T).astype(NP_BF16) for b in range(B)]
    woT = np.ascontiguousarray(Wo.T).astype(NP_BF16)
    diag = np.where(np.arange(128)[:, None] > np.arange(128)[None, :],
                    np.float32(NEG), np.float32(0.0)).astype(np.float32)

    si = np.arange(S, dtype=np.float64)
    p = np.arange(128, dtype=np.float64)

    in_maps = []
    for c in range(N_CORES):
        hs = [2 * c, 2 * c + 1]
        m = {}
        for b in range(B):
            m[f"xT{b}"] = xT[b]
        for name, W in (("wq", Wq), ("wk", Wk), ("wv", Wv)):
            m[name] = np.ascontiguousarray(
                W[hs[0] * D:(hs[1] + 1) * D, :].T).astype(NP_BF16)
        m["woT"] = woT
        bias = np.empty((128, HPC * ST_TILES), np.float32)
        br = np.empty((1, HPC * S), NP_BF16)
        for i, h in enumerate(hs):
            sl = slopes[h]
            for tj in range(ST_TILES):
                bias[:, i * ST_TILES + tj] = (sl * (tj * 128 + p)).astype(
                    np.float32)
            br[0, i * S:(i + 1) * S] = (-sl * si / SCALE).astype(NP_BF16)
        m["bias_sj"] = bias
        m["brow"] = br
        m["diag"] = diag
        in_maps.append(m)
    return in_maps


def _run(inputs, trace=False):
    nc = _get_nc()
    in_maps = _make_in_maps(inputs["x"], inputs["Wq"], inputs["Wk"],
                            inputs["Wv"], inputs["Wo"])
    last_err = None
    for attempt in range(3):
        try:
            res = run_bass_kernel_spmd(nc, in_maps,
                                       core_ids=list(range(N_CORES)),
                                       trace=trace)
            break
        except Exception as e:  # transient NRT device errors; retry
            last_err = e
            if "UNRECOVERABLE" not in str(e) and "UNAVAILABLE" not in str(e):
                raise
    else:
        raise last_err
    out = np.empty((B, S, E), np.float32)
    for c in range(N_CORES):
        b, r = c // 4, c % 4
        out[b, r * 512:(r + 1) * 512, :] = res.results[c]["out"]
    # bv shifts the attention output by a constant vector (P rows sum to 1
    # after normalization), so it folds into a constant output-row shift
    # through Wo; bo adds directly. bq/bk are zeros per the problem spec
    # (bk would cancel in softmax anyway; bq is assumed zero).
    shift = inputs["bv"].astype(np.float32) @ inputs["Wo"].T.astype(np.float32)
    out += (shift + inputs["bo"].astype(np.float32))[None, None, :]
    return out, res


def kernel(**inputs) -> np.ndarray:
    out, _ = _run(inputs, trace=False)
    return out


# revision 50
# speedup vs baseline: 1.0226x; 1.0226x over previous
r"""ALiBi multi-head causal attention on 8 TRN2 NeuronCores.

Problem: B=2, S=2048, E=2048, H=16, D=128, f32 I/O.

Sharding: each core owns 2 heads (core c -> heads {2c, 2c+1}) for BOTH
batches. Per core: QKV projections for its heads, causal attention with
ALiBi, then one 8-core AllToAll per local head re-shards from head-split
to row-split (the first collective overlaps the second head's attention,
the second overlaps the first half of the output projection), and each
core computes the output projection for its (batch, row-slice):
core c -> batch c//4, rows (c%4)*512 .. +512.

Compute dtype bf16 (fp32 PSUM accumulation). Numerical scheme for the
softmax (scores are built TRANSPOSED: ST[sj, si] so the P@V contraction
needs no on-chip transpose of P):

  exp argument = scale*qk[sj,si] + slope*sj  + scale*brow[si]
                 \__ matmul __/   \_ACT bias_/  \_ K=128 matmul _/

where brow[si] = bf16(-slope*si/scale) is replicated across partitions
and added via a K=128 matmul against a constant 1/128 stationary operand
(exact, and keeps the PE MAC-dense so the HAM clock gate stays open).

The row max is never computed: with x ~ N(0,1) and W ~ N(0,1/E) the raw
qk*scale scores are bounded (~±60), so exp never overflows fp32/bf16, and
the per-row (si) shift -slope*si only needs to be applied approximately:
any per-si error is a multiplicative per-row factor that cancels exactly
in the softmax normalization (rowsum is obtained via a ones-column
appended to V). The causal mask is a -1e9 additive [128,128] triangle on
the diagonal tiles; fully-masked tiles are simply never computed.

Schedule: score strips (ACT-bound) are interleaved at matmul granularity
with the previous iteration's P@V groups (PE-dense); normalized O tiles
are staged straight into the A2A input buffers (one DMA per row tile);
the transposed ZT loads ([si,d] -> [d,si] via the DMA xbar) are pinned
behind the staging DMAs so a collective-wait never blocks the sync queue.
"""

import math

import numpy as np
import ml_dtypes

import concourse.bass as bass
import concourse.tile as tile
from concourse.tile import add_dep_helper
from concourse import bacc, mybir
from concourse.bass_utils import run_bass_kernel_spmd

BF16 = mybir.dt.bfloat16
F32 = mybir.dt.float32
NP_BF16 = ml_dtypes.bfloat16

B, S, E, H, D = 2, 2048, 2048, 16, 128
N_CORES = 8
HPC = H // N_CORES  # heads per core = 2
SCALE = 1.0 / math.sqrt(D)
ET = E // 128  # 16 e-tiles
ST_TILES = S // 128  # 16 sequence tiles
NEG = -1.0e9


def _slopes():
    # H=16 is a power of two: slope_i = 2^(-(i+1)/2)
    start = 2.0 ** (-(2.0 ** -(math.log2(H) - 3)))
    return [start * start**i for i in range(H)]


def _build():
    nc = bacc.Bacc("TRN2", target_bir_lowering=False, debug=False,
                   num_devices=N_CORES)

    xT = [nc.dram_tensor(f"xT{b}", [E, S], BF16, kind="ExternalInput")
          for b in range(B)]
    wq = nc.dram_tensor("wq", [E, HPC * D], BF16, kind="ExternalInput")
    wk = nc.dram_tensor("wk", [E, HPC * D], BF16, kind="ExternalInput")
    wv = nc.dram_tensor("wv", [E, HPC * D], BF16, kind="ExternalInput")
    woT = nc.dram_tensor("woT", [E, E], BF16, kind="ExternalInput")
    bias_sj = nc.dram_tensor("bias_sj", [128, HPC * ST_TILES], F32,
                             kind="ExternalInput")
    brow = nc.dram_tensor("brow", [1, HPC * S], BF16, kind="ExternalInput")
    diag = nc.dram_tensor("diag", [128, 128], F32, kind="ExternalInput")
    out_ext = nc.dram_tensor("out", [512, E], F32, kind="ExternalOutput")

    # A2A (one per local head hh, so the first collective overlaps the
    # second head's attention): chunk j (sent to core j) = head hh's
    # normalized attention output [512(si), 128(d)] for core j's
    # (batch j//4, rows (j%4)*512..+512). After A2A, chunk j = head
    # 2j+hh for MY (batch, rows) slice; transposed to [d, si] by the
    # xbar on the DRAM->SBUF load.
    a2a_in = [nc.dram_tensor(f"a2a_in{h}", [N_CORES, 512, D], BF16)
              for h in range(HPC)]
    a2a_out = [nc.dram_tensor(f"a2a_out{h}", [N_CORES, 512, D], BF16)
               for h in range(HPC)]

    with tile.TileContext(nc) as tc:
        _emit(nc, tc, xT, wq, wk, wv, woT, bias_sj, brow, diag,
              a2a_in, a2a_out, out_ext)

    nc.compile()
    return nc


def _emit(nc, tc, xT, wq, wk, wv, woT, bias_sj, brow, diag,
          a2a_in, a2a_out, out_ext):
    from contextlib import ExitStack

    with ExitStack() as top:
        consts = top.enter_context(tc.tile_pool(name="consts", bufs=1))
        qkv = top.enter_context(tc.tile_pool(name="qkv", bufs=1))
        proj_psum = top.enter_context(
            tc.tile_pool(name="proj_psum", bufs=3, space="PSUM"))

        # ---- constants ----
        sb_bias = consts.tile([128, HPC * ST_TILES], F32, tag="bias")
        nc.gpsimd.dma_start(out=sb_bias[:], in_=bias_sj[:])
        # brow replicated across all 128 partitions: the per-si ALiBi shift
        # is added via a K=128 matmul with a constant 1/128 stationary
        # operand (exact: 1/128 is a power of two) -- same cycle count as
        # a rank-1 but with full MAC activity so the HAM clock gate stays
        # released during the score phase.
        sb_brow = consts.tile([128, HPC * S], BF16, tag="brow")
        nc.gpsimd.dma_start(
            out=sb_brow[:],
            in_=bass.AP(tensor=brow.ap().tensor, offset=0,
                        ap=[[0, 128], [1, HPC * S]]))
        sb_diag = consts.tile([128, 128], F32, tag="diag")
        nc.gpsimd.dma_start(out=sb_diag[:], in_=diag[:])
        sb_ones = consts.tile([128, 128], BF16, tag="ones")
        nc.vector.memset(sb_ones[:], 1.0 / 128.0)

        # ---- persistent QKV buffers ----
        # QT/KT: [128(d), B, HPC, S];  V: [128(s), ST_TILES, B, HPC, 132]
        QT = qkv.tile([128, B, HPC, S], BF16, tag="QT")
        KT = qkv.tile([128, B, HPC, S], BF16, tag="KT")
        V = qkv.tile([128, ST_TILES, B, HPC, 132], BF16, tag="V")
        nc.vector.memset(V[:, :, :, :, 128:129], 1.0)  # rowsum ones column

        # ---- phase 1: QKV projections ----
        with tc.tile_pool(name="wbuf", bufs=1) as wbuf, \
             tc.tile_pool(name="xtp", bufs=3) as xtp:
            w_sb = {}
            for name, wt in (("q", wq), ("k", wk), ("v", wv)):
                w_sb[name] = wbuf.tile([128, ET, HPC * D], BF16,
                                       tag=f"w{name}", name=f"w{name}")
            w_r = {name: wt.ap().rearrange("(t p) c -> p t c", p=128)
                   for name, wt in (("q", wq), ("k", wk), ("v", wv))}
            xT_r0 = xT[0].ap().rearrange("(t p) s -> p t s", p=128)
            xt_first = xtp.tile([128, ET, 512], BF16, name="xt_t")
            # head of the DMA queue: interleave wq with the first x
            # chunk so the very first matmul group can start ~6us in
            for q in range(8):
                w = 2
                nc.sync.dma_start(
                    out=w_sb["q"][:, w * q:w * (q + 1), :],
                    in_=w_r["q"][:, w * q:w * (q + 1), :])
                nc.sync.dma_start(
                    out=xt_first[:, w * q:w * (q + 1), :],
                    in_=xT_r0[:, w * q:w * (q + 1), 0:512])
            for name in ("k", "v"):
                for q in range(4):
                    nc.sync.dma_start(
                        out=w_sb[name][:, 4 * q:4 * (q + 1), :],
                        in_=w_r[name][:, 4 * q:4 * (q + 1), :])

            for b in range(B):
                xT_r = xT[b].ap().rearrange("(t p) s -> p t s", p=128)
                for sc in range(4):  # s-chunks of 512
                    if (b, sc) == (0, 0):
                        xt_t = xt_first
                    else:
                        xt_t = xtp.tile([128, ET, 512], BF16, name="xt_t")
                        for q in range(2):
                            w = ET // 2
                            nc.sync.dma_start(
                                out=xt_t[:, w * q:w * (q + 1), :],
                                in_=xT_r[:, w * q:w * (q + 1),
                                         sc * 512:(sc + 1) * 512])
                    # QT / KT (transposed layout; m = head-dim)
                    for name, OUT in (("q", QT), ("k", KT)):
                        for hh in range(HPC):
                            ps = proj_psum.tile([128, 512], F32, tag="ps")
                            for et in range(ET):
                                nc.tensor.matmul(
                                    ps[:],
                                    lhsT=w_sb[name][:, et,
                                                    hh * 128:(hh + 1) * 128],
                                    rhs=xt_t[:, et, :],
                                    start=(et == 0), stop=(et == ET - 1))
                            nc.vector.tensor_copy(
                                out=OUT[:, b, hh, sc * 512:(sc + 1) * 512],
                                in_=ps[:])
                    # V (natural layout; m = sequence)
                    for mt in range(4):
                        ps = proj_psum.tile([128, 512], F32, tag="ps")
                        for et in range(ET):
                            nc.tensor.matmul(
                                ps[:, :HPC * D],
                                lhsT=xt_t[:, et, mt * 128:(mt + 1) * 128],
                                rhs=w_sb["v"][:, et, :],
                                start=(et == 0), stop=(et == ET - 1))
                        st = sc * 4 + mt
                        nc.vector.tensor_copy(
                            out=V[:, st, b, :, 0:128],
                            in_=ps[:, :HPC * D].rearrange(
                                "p (h d) -> p h d", h=HPC))

        # ---- phase 2: attention ----
        with tc.tile_pool(name="ptp", bufs=30) as ptp, \
             tc.tile_pool(name="onp", bufs=8) as onp, \
             tc.tile_pool(name="rcp", bufs=8) as rcp, \
             tc.tile_pool(name="wob", bufs=1) as wob, \
             tc.tile_pool(name="ztb", bufs=1) as ztb, \
             tc.tile_pool(name="y0b", bufs=1) as y0b, \
             tc.tile_pool(name="outp", bufs=3) as outp, \
             tc.tile_pool(name="st_psum", bufs=3, space="PSUM") as st_psum, \
             tc.tile_pool(name="o_psum", bufs=2, space="PSUM") as o_psum:

            ZT = ztb.tile([128, HPC, N_CORES, 512], BF16)
            Y0 = y0b.tile([128, 4, E], BF16)  # even-head partial out proj
            wo_sb = wob.tile([128, ET, E], BF16)
            woT_r = woT.ap().rearrange("(t p) e -> p t e", p=128)
            for q in range(4):  # bulk load off the latency-critical queue
                nc.gpsimd.dma_start(
                    out=wo_sb[:, 4 * q:4 * (q + 1), :],
                    in_=woT_r[:, 4 * q:4 * (q + 1), :])

            def emit_one_strip(b, hh, sb, tj):
                si_lo = max(sb * 512, tj * 128)
                n = (sb + 1) * 512 - si_lo
                ps = st_psum.tile([128, 512], F32, tag="st")
                nc.tensor.matmul(
                    ps[:, :n],
                    lhsT=KT[:, b, hh, tj * 128:(tj + 1) * 128],
                    rhs=QT[:, b, hh, si_lo:si_lo + n],
                    start=True, stop=False)
                nc.tensor.matmul(
                    ps[:, :n],
                    lhsT=sb_ones[:, :],
                    rhs=sb_brow[:, hh * S + si_lo:hh * S + si_lo + n],
                    start=False, stop=True)  # adds brow (K=128, x 1/128)
                if tj >= 4 * sb:  # diagonal tile: causal mask
                    nc.vector.tensor_add(
                        ps[:, 0:128], ps[:, 0:128], sb_diag[:])
                strip = ptp.tile([128, 512], BF16)
                last_exp[0] = nc.scalar.activation(
                    strip[:, :n], ps[:, :n],
                    mybir.ActivationFunctionType.Exp,
                    bias=sb_bias[:, hh * ST_TILES + tj:
                                 hh * ST_TILES + tj + 1],
                    scale=SCALE)
                return (si_lo, strip)

            def pv_gen(hh, b, sb, strips):
                """Yield after each PV matmul; norm + stage between."""
                for ti in range(4 * sb, 4 * sb + 4):
                    op = o_psum.tile([128, 132], F32, name="op")
                    for tj in range(ti + 1):
                        si_lo, strip = strips[tj]
                        col = ti * 128 - si_lo
                        nc.tensor.matmul(
                            op[:, 0:129],
                            lhsT=strip[:, col:col + 128],
                            rhs=V[:, tj, b, hh, 0:129],
                            start=(tj == 0), stop=(tj == ti))
                        yield
                    recip = rcp.tile([128, 1], F32, name="recip")
                    nc.vector.reciprocal(recip[:], op[:, 128:129])
                    onorm = onp.tile([128, 128], BF16, name="onorm")
                    nc.vector.tensor_scalar_mul(
                        onorm[:], op[:, 0:128], recip[:])
                    # stage this row-tile of the A2A chunk for core 4*b+sb
                    dst = a2a_in[hh][4 * b + sb,
                                     (ti % 4) * 128:(ti % 4 + 1) * 128, :]
                    last_stage[0] = nc.sync.dma_start(out=dst, in_=onorm[:])
                    yield

            def emit_a2a(hh):
                nc.gpsimd.collective_compute(
                    "AllToAll",
                    mybir.AluOpType.bypass,
                    ins=[a2a_in[hh].ap().opt()],
                    outs=[a2a_out[hh].ap().opt()],
                    replica_groups=[list(range(N_CORES))],
                )

            def emit_zt(hh, anchor):
                # xbar-transposing load ([si, d] -> [d, si]). Pinned after
                # `anchor`: a collective-wait sitting mid-queue would stall
                # the latency-critical staging DMAs behind it. NOTE: all
                # transposing DMAs stay on the single sync queue -- issuing
                # them concurrently from two HWDGE queues was observed to
                # corrupt data (the xbar-mode serialization that protects
                # DMATranspose/DMACopy transitions is per-queue only).
                inst = nc.sync.dma_start(
                    out=ZT[:, hh, :, :],
                    in_=a2a_out[hh].ap().rearrange("j s d -> (j s) d"),
                    transpose=True)
                if anchor is not None:
                    add_dep_helper(inst.ins, anchor.ins, sync=False,
                                   reason="zt ordering")
                return inst

            def emit_outproj(rnd):
                # rnd 0: even c-tiles (head hh=0 of each core) -> Y0;
                # runs while the second head's collectives are in flight.
                # rnd 1: odd c-tiles + Y0 -> out; mt-outer so row tile mt
                # only waits for si-quarter mt of the A2A.
                cts = [ct for ct in range(ET) if ct % HPC == rnd]
                loop = ([(ec, mt) for ec in range(4) for mt in range(4)]
                        if rnd == 0 else
                        [(ec, mt) for mt in range(4) for ec in range(4)])
                for ec, mt in loop:
                    if True:
                        ps = proj_psum.tile([128, 512], F32, name="ps3", tag="ps")
                        for k, ct in enumerate(cts):
                            nc.tensor.matmul(
                                ps[:],
                                lhsT=ZT[:, ct % HPC, ct // HPC,
                                        mt * 128:(mt + 1) * 128],
                                rhs=wo_sb[:, ct, ec * 512:(ec + 1) * 512],
                                start=(k == 0), stop=(k == len(cts) - 1))
                        if rnd == 0:
                            last_y0[0] = nc.scalar.copy(
                                out=Y0[:, mt, ec * 512:(ec + 1) * 512],
                                in_=ps[:])
                        else:
                            ot = outp.tile([128, 512], F32, name="ot")
                            nc.vector.tensor_add(
                                ot[:], ps[:],
                                Y0[:, mt, ec * 512:(ec + 1) * 512])
                            nc.sync.dma_start(
                                out=out_ext[mt * 128:(mt + 1) * 128,
                                            ec * 512:(ec + 1) * 512],
                                in_=ot[:])

            # software pipeline: iteration i's score strips (ACT-bound,
            # PE-light) are interleaved at matmul granularity with
            # iteration i-1's PV matmuls (PE-dense), so the PE never
            # crawls at the exp rate and the HAM clock gate stays warm.
            iters = [(hh, b, sb)
                     for hh in range(HPC) for b in range(B)
                     for sb in range(4)]
            last_stage = [None]
            zt0_anchor = [None]
            last_exp = [None]
            last_y0 = [None]
            pending = None  # (hh, b, sb, generator)
            for hh, b, sb in iters:
                nstrips = 4 * sb + 4
                strips = {}
                if pending is not None:
                    phh, pb, psb, pgen = pending
                    # prev iteration has 16*psb+14 pv ops; spread them
                    per = (16 * psb + 14 + nstrips - 1) // nstrips + 1
                for tj in range(nstrips):
                    strips[tj] = emit_one_strip(b, hh, sb, tj)
                    if pending is not None:
                        for _ in range(per):
                            if next(pgen, "done") == "done":
                                break
                if pending is not None:
                    for _ in pgen:  # flush remainder
                        pass
                    if pb == B - 1 and psb == 3:
                        emit_a2a(phh)
                    if phh == HPC - 1 and pb == B - 1 and psb == 2:
                        zt0_anchor[0] = last_stage[0]
                pending = (hh, b, sb, pv_gen(hh, b, sb, strips))
            phh, pb, psb, pgen = pending
            for _ in pgen:
                pass
            emit_a2a(phh)

            # ---- phase 3: output projection (split rounds) ----
            zt0 = emit_zt(0, last_stage[0])
            emit_outproj(0)
            emit_zt(1, zt0)
            emit_outproj(1)


_NC_CACHE = None


def _get_nc():
    global _NC_CACHE
    if _NC_CACHE is None:
        _NC_CACHE = _build()
    return _NC_CACHE


def _make_in_maps(x, Wq, Wk, Wv, Wo):
    slopes = _slopes()
    xT = [np.ascontiguousarray(x[b].T).astype(NP_BF16) for b in range(B)]
    woT = np.ascontiguousarray(Wo.T).astype(NP_BF16)
    diag = np.where(np.arange(128)[:, None] > np.arange(128)[None, :],
                    np.float32(NEG), np.float32(0.0)).astype(np.float32)

    si = np.arange(S, dtype=np.float64)
    p = np.arange(128, dtype=np.float64)

    in_maps = []
    for c in range(N_CORES):
        hs = [2 * c, 2 * c + 1]
        m = {}
        for b in range(B):
            m[f"xT{b}"] = xT[b]
        for name, W in (("wq", Wq), ("wk", Wk), ("wv", Wv)):
            m[name] = np.ascontiguousarray(
                W[hs[0] * D:(hs[1] + 1) * D, :].T).astype(NP_BF16)
        m["woT"] = woT
        bias = np.empty((128, HPC * ST_TILES), np.float32)
        br = np.empty((1, HPC * S), NP_BF16)
        for i, h in enumerate(hs):
            sl = slopes[h]
            for tj in range(ST_TILES):
                bias[:, i * ST_TILES + tj] = (sl * (tj * 128 + p)).astype(
                    np.float32)
            br[0, i * S:(i + 1) * S] = (-sl * si / SCALE).astype(NP_BF16)
        m["bias_sj"] = bias
        m["brow"] = br
        m["diag"] = diag
        in_maps.append(m)
    return in_maps


def _run(inputs, trace=False):
    nc = _get_nc()
    in_maps = _make_in_maps(inputs["x"], inputs["Wq"], inputs["Wk"],
                            inputs["Wv"], inputs["Wo"])
    last_err = None
    for attempt in range(3):
        try:
            res = run_bass_kernel_spmd(nc, in_maps,
                                       core_ids=list(range(N_CORES)),
                                       trace=trace)
            break
        except Exception as e:  # transient NRT device errors; retry
            last_err = e
            if "UNRECOVERABLE" not in str(e) and "UNAVAILABLE" not in str(e):
                raise
    else:
        raise last_err
    out = np.empty((B, S, E), np.float32)
    for c in range(N_CORES):
        b, r = c // 4, c % 4
        out[b, r * 512:(r + 1) * 512, :] = res.results[c]["out"]
    # bv shifts the attention output by a constant vector (P rows sum to 1
    # after normalization), so it folds into a constant output-row shift
    # through Wo; bo adds directly. bq/bk are zeros per the problem spec
    # (bk would cancel in softmax anyway; bq is assumed zero).
    shift = inputs["bv"].astype(np.float32) @ inputs["Wo"].T.astype(np.float32)
    out += (shift + inputs["bo"].astype(np.float32))[None, None, :]
    return out, res


def kernel(**inputs) -> np.ndarray:
    out, _ = _run(inputs, trace=False)
    return out
